# revision 16
# baseline (speedup 1.0000x reference)
"""Trainium2 Bass kernel for nn_MatchingLayer (segment_reduce).

Strategy (data-parallel over batch B=8, one batch element per NeuronCore):
  - The heavy op is, per pair, a max over a dynamic window (up to 29x29 cells,
    each cell = 768 contiguous f32) of Table[b].
  - Host enumerates each pair's window cells, chunks them into 16-cell "units",
    and lays units over (lane, step) slots -> perfectly load-balanced across
    128 lanes regardless of per-pair window size.
  - Stage 1 (device): per step, dma_gather 16 cells/lane (2048 cells, ~6 MB),
    strided reduce_max over the 16 slots -> one 768-vec partial per unit,
    spilled to a DRAM scratch.
  - Stage 2 (device): pairs sorted by unit count into 2 tiles of 128; gather
    each pair's partials (pad row = -1e30 for unused slots) and reduce to
    R[pair, 768].
  - S/E/S2/E2 single-cell features are gathered straight into the feats tile;
  - logits = feats . W^T via 4 tensor_tensor_reduce dot products per tile.
  - Host: unsort, +bias, softmax/CE (tiny: 2048x4) and the scalar-loss
    "all-reduce" across cores.
"""

import os
import numpy as np

import concourse.bacc as bacc
import concourse.mybir as mybir
from concourse.bass_utils import run_bass_kernel_spmd
from concourse.tile import TileContext

B, L, H, P = 8, 128, 768, 256
WMAX = 30
NCELL = L * L          # 16384 cells per table
PAD1 = NCELL           # pad row index in padded table
C1 = 16                # cells per unit (stage-1 slots per lane per step)
C2 = 8                 # partials per lane per step in stage 2
NUM1 = 128 * C1        # idx per stage-1 gather
NUM2 = 128 * C2
NUMF = 128 * 2         # idx per feature gather (2 cells per pair)

_f32 = mybir.dt.float32
_bf16 = mybir.dt.bfloat16
_i16 = mybir.dt.int16


def _pack_idx(flat):
    """flat [NUM] int -> wrapped-16 idx layout [128, NUM//16] int16,
    replicated across the 8 groups of 16 partitions."""
    n = len(flat)
    assert n % 16 == 0
    a = np.asarray(flat, np.int16).reshape(-1, 16).T  # [16, n//16]
    return np.tile(a, (8, 1))  # [128, n//16]


def _plan_core(pairs_b):
    """Host planning for one core. Returns dict of idx arrays + metadata."""
    s0, e0, s1, e1 = (pairs_b[:, i].astype(np.int64) for i in range(4))
    asp = s0 < 0
    r0 = np.where(asp, s1, s0)
    q0 = np.where(asp, e1, e0)
    nr = q0 - r0                      # window rows  (>=1)
    nc = e1 - s1                      # window cols  (>=1)
    a = r0 + 1                        # first row
    c0 = s1 + 1                       # first col

    # stage 1: per-pair cell lists chunked into 16-cell units
    unit_cells = []                   # list of [16] int arrays (PAD1-padded)
    unit_pair = []                    # owning pair of each unit
    for p in range(P):
        rows = np.arange(a[p], a[p] + nr[p])
        cols = np.arange(c0[p], c0[p] + nc[p])
        cells = (rows[:, None] * L + cols[None, :]).ravel()
        m = (len(cells) + C1 - 1) // C1
        padded = np.full(m * C1, PAD1, np.int64)
        padded[:len(cells)] = cells
        for u in range(m):
            unit_cells.append(padded[u * C1:(u + 1) * C1])
            unit_pair.append(p)
    nunits = len(unit_cells)

    # feature cells
    Scell = (r0 + 1) * L + (s1 + 1)
    Ecell = q0 * L + e1
    S2cell = ((s0 + 1) % L) * L + (s1 + 1)
    E2cell = (e0 % L) * L + e1

    return dict(nunits=nunits, unit_cells=unit_cells, unit_pair=unit_pair,
                Scell=Scell, Ecell=Ecell, S2cell=S2cell, E2cell=E2cell)


def _finish_plan(plan, G1, caps):
    """Layout stage-1 units onto (lane, step) slots and build all idx arrays,
    given the cross-core maxima G1 (stage-1 steps) and caps (per-tile stage-2
    unit capacity)."""
    nunits = plan["nunits"]
    PAD2 = G1 * 128                   # pad row in partials scratch

    # stage-1 idx: unit u -> lane u%128, step u//128
    idx1 = np.full((G1, NUM1), PAD1, np.int64)
    prow = np.empty(nunits, np.int64)     # partial row of unit u
    for u, cells in enumerate(plan["unit_cells"]):
        lane, step = u % 128, u // 128
        # flat slot (p=lane, c) -> i = c*128 + p
        idx1[step, lane::128] = cells
        prow[u] = step * 128 + lane
    idx1_packed = np.concatenate([_pack_idx(idx1[g]) for g in range(G1)], axis=1)

    # per-pair partial lists
    plist = [[] for _ in range(P)]
    for u, p in enumerate(plan["unit_pair"]):
        plist[p].append(prow[u])
    m = np.array([len(x) for x in plist])

    # sort pairs by unit count desc; tile0 = big pairs
    order = np.argsort(-m, kind="stable")
    idx2_packed_tiles = []
    G2s = []
    for t in range(2):
        tp = order[t * 128:(t + 1) * 128]
        cap = caps[t]
        G2 = (cap + C2 - 1) // C2
        G2s.append(G2)
        idx2 = np.full((G2, NUM2), PAD2, np.int64)
        for lane, p in enumerate(tp):
            for c, pr in enumerate(plist[p]):
                idx2[c // C2, (c % C2) * 128 + lane] = pr
        idx2_packed_tiles.append(
            np.concatenate([_pack_idx(idx2[g]) for g in range(G2)], axis=1))

    # feature idx in sorted order, per tile: [S|E] from table, [S2|E2] from edge
    fidx_t, fidx_e = [], []
    for t in range(2):
        tp = order[t * 128:(t + 1) * 128]
        se = np.concatenate([plan["Scell"][tp], plan["Ecell"][tp]])
        s2e2 = np.concatenate([plan["S2cell"][tp], plan["E2cell"][tp]])
        fidx_t.append(_pack_idx(se))
        fidx_e.append(_pack_idx(s2e2))

    return dict(order=order, m=m,
                idx1=idx1_packed,
                idx2=np.concatenate(idx2_packed_tiles, axis=1),
                fidx=np.concatenate(fidx_t + fidx_e, axis=1))


def _build_program(G1, G2s):
    nc = bacc.Bacc(name="matching_layer")
    tabp_d = nc.dram_tensor("tabp", [NCELL + 1, H], _f32, kind="ExternalInput")
    edge_d = nc.dram_tensor("edge", [NCELL, H], _f32, kind="ExternalInput")
    idx1_d = nc.dram_tensor("idx1", [128, G1 * NUM1 // 16], _i16, kind="ExternalInput")
    idx2_d = nc.dram_tensor("idx2", [128, sum(G2s) * NUM2 // 16], _i16, kind="ExternalInput")
    fidx_d = nc.dram_tensor("fidx", [128, 4 * NUMF // 16], _i16, kind="ExternalInput")
    wrep_d = nc.dram_tensor("wrep", [128, 4 * 5 * H], _f32, kind="ExternalInput")
    out_d = nc.dram_tensor("logits", [P, 128], _f32, kind="ExternalOutput")

    NPROW = G1 * 128 + 1   # partial rows + pad row

    with TileContext(nc) as tc:
        with tc.tile_pool(name="dram", bufs=1, space="DRAM") as dpool, \
             tc.tile_pool(name="const", bufs=1) as cpool:

            partials = dpool.tile([NPROW, H], _bf16)
            tab16 = dpool.tile([NCELL + 1, H], _bf16)

            # one-time f32 -> bf16 table conversion (SWDGE cast, no DVE work)
            with tc.tile_pool(name="cvt", bufs=3) as cvtpool:
                CROWS = 512
                for r in range(0, NCELL + 1, CROWS):
                    n = min(CROWS, NCELL + 1 - r)
                    ct = cvtpool.tile([128, (CROWS // 128) * H], _bf16, tag="cvt")
                    src = tabp_d[r:r + n, :].rearrange("(a b) h -> a (b h)", a=128) \
                        if n == CROWS else None
                    if n == CROWS:
                        nc.gpsimd.dma_start(out=ct[:], in_=src)
                        nc.sync.dma_start(
                            out=tab16[r:r + n, :].rearrange("(a b) h -> a (b h)", a=128),
                            in_=ct[:])
                    else:
                        ct2 = cvtpool.tile([n, H], _bf16, tag="cvt2")
                        nc.gpsimd.dma_start(out=ct2[:], in_=tabp_d[r:r + n, :])
                        nc.sync.dma_start(out=tab16[r:r + n, :], in_=ct2[:])

            # preload all idx
            idx1_t = cpool.tile([128, G1 * NUM1 // 16], _i16)
            nc.sync.dma_start(out=idx1_t[:], in_=idx1_d[:])
            idx2_t = cpool.tile([128, sum(G2s) * NUM2 // 16], _i16)
            nc.sync.dma_start(out=idx2_t[:], in_=idx2_d[:])
            fidx_t = cpool.tile([128, 4 * NUMF // 16], _i16)
            nc.sync.dma_start(out=fidx_t[:], in_=fidx_d[:])

            # partials pad row = -1e30
            padrow = cpool.tile([1, H], _bf16)
            nc.vector.memset(padrow[:], -1e30)
            nc.sync.dma_start(out=partials[NPROW - 1:NPROW, :], in_=padrow[:])

            # ---- stage 1: gather 16 cells/lane, reduce, spill partials ----
            W1 = NUM1 // 16
            with tc.tile_pool(name="s1", bufs=1) as s1pool:
                halfW = W1 // 2
                for g in range(G1):
                    slot = s1pool.tile([128, C1, H], _bf16, tag="slot1", bufs=3)
                    # dma_gather is limited to 1024 indices per instruction
                    nc.gpsimd.dma_gather(slot[:, 0:C1 // 2, :], tab16[:],
                                         idx1_t[:, g * W1:g * W1 + halfW],
                                         NUM1 // 2, NUM1 // 2, H)
                    nc.gpsimd.dma_gather(slot[:, C1 // 2:C1, :], tab16[:],
                                         idx1_t[:, g * W1 + halfW:(g + 1) * W1],
                                         NUM1 // 2, NUM1 // 2, H)
                    # contiguous bf16 TT-max tree: 16 -> 1 slots
                    w = C1 // 2
                    while w >= 1:
                        nc.vector.tensor_tensor(
                            out=slot[:, 0:w, :], in0=slot[:, 0:w, :],
                            in1=slot[:, w:2 * w, :], op=mybir.AluOpType.max)
                        w //= 2
                    nc.sync.dma_start(out=partials[g * 128:(g + 1) * 128, :],
                                      in_=slot[:, 0, :])

            # ---- stage 2 + features + logits per tile ----
            W2 = NUM2 // 16
            WF = NUMF // 16
            kstages = int(os.environ.get("KSTAGES", "3"))
            with tc.tile_pool(name="s2", bufs=1) as s2pool:
                wrep_t = s2pool.tile([128, 4 * 5 * H], _f32, bufs=1)
                nc.sync.dma_start(out=wrep_t[:], in_=wrep_d[:])
                g2_off = 0
                for t in range(2 if kstages >= 2 else 0):
                    feats = s2pool.tile([128, 5 * H], _f32, tag="feats", bufs=2)
                    # R = max over this tile's stage-2 steps
                    for s in range(G2s[t]):
                        slot2 = s2pool.tile([128, C2, H], _bf16, tag="slot2", bufs=2)
                        nc.gpsimd.dma_gather(
                            slot2[:], partials[:],
                            idx2_t[:, (g2_off + s) * W2:(g2_off + s + 1) * W2],
                            NUM2, NUM2, H)
                        w = C2 // 2
                        while w >= 2:
                            nc.vector.tensor_tensor(
                                out=slot2[:, 0:w, :], in0=slot2[:, 0:w, :],
                                in1=slot2[:, w:2 * w, :], op=mybir.AluOpType.max)
                            w //= 2
                        if s == 0:
                            nc.vector.tensor_tensor(
                                out=feats[:, 2 * H:3 * H], in0=slot2[:, 0, :],
                                in1=slot2[:, 1, :], op=mybir.AluOpType.max)
                        else:
                            rtmp = s2pool.tile([128, H], _f32, tag="rtmp", bufs=2)
                            nc.vector.tensor_tensor(
                                out=rtmp[:], in0=slot2[:, 0, :],
                                in1=slot2[:, 1, :], op=mybir.AluOpType.max)
                            nc.vector.tensor_tensor(
                                out=feats[:, 2 * H:3 * H],
                                in0=feats[:, 2 * H:3 * H],
                                in1=rtmp[:], op=mybir.AluOpType.max)
                    g2_off += G2s[t]

                    # S,E then S2,E2
                    nc.gpsimd.dma_gather(
                        feats[:, 0:2 * H].rearrange("p (c h) -> p c h", h=H),
                        tabp_d[:], fidx_t[:, t * WF:(t + 1) * WF], NUMF, NUMF, H)
                    nc.gpsimd.dma_gather(
                        feats[:, 3 * H:5 * H].rearrange("p (c h) -> p c h", h=H),
                        edge_d[:], fidx_t[:, (2 + t) * WF:(3 + t) * WF],
                        NUMF, NUMF, H)

                    if kstages < 3:
                        continue
                    ltile = s2pool.tile([128, 128], _f32, tag="ltile", bufs=2)
                    nc.vector.memset(ltile[:], 0.0)
                    scratch = s2pool.tile([128, 5 * H], _f32, tag="scratch", bufs=1)
                    for c in range(4):
                        nc.vector.tensor_tensor(
                            out=scratch[:], in0=feats[:],
                            in1=wrep_t[:, c * 5 * H:(c + 1) * 5 * H],
                            op=mybir.AluOpType.mult)
                        nc.vector.tensor_reduce(
                            out=ltile[:, c:c + 1], in_=scratch[:],
                            axis=mybir.AxisListType.X, op=mybir.AluOpType.add)
                    if int(os.environ.get("KOUT", "1")):
                        nc.sync.dma_start(out=out_d[t * 128:(t + 1) * 128, :],
                                          in_=ltile[:])

    nc.finalize()
    return nc


def kernel(Table, table_edge, pairs, labels, W, b):
    Table = np.ascontiguousarray(np.asarray(Table, np.float32))
    table_edge = np.ascontiguousarray(np.asarray(table_edge, np.float32))
    pairs = np.asarray(pairs, np.int32)
    labels = np.asarray(labels, np.int32)
    W = np.asarray(W, np.float32)
    bb = np.asarray(b, np.float32)

    plans = [_plan_core(pairs[i]) for i in range(B)]
    G1 = max((pl["nunits"] + 127) // 128 for pl in plans)

    # stage-2 per-tile capacity = max unit count among that tile's pairs
    caps = [0, 0]
    for pl in plans:
        mm = np.zeros(P, np.int64)
        for u, p in enumerate(pl["unit_pair"]):
            mm[p] += 1
        ms = np.sort(mm)[::-1]
        caps[0] = max(caps[0], int(ms[0]))
        caps[1] = max(caps[1], int(ms[128]))
    G2s = [(caps[0] + C2 - 1) // C2, (caps[1] + C2 - 1) // C2]

    fins = [_finish_plan(pl, G1, caps) for pl in plans]

    nc = _build_program(G1, G2s)

    wrep = np.tile(W.reshape(1, 4 * 5 * H), (128, 1)).astype(np.float32)
    padrow = np.full((1, H), -1e30, np.float32)
    in_maps = []
    for i in range(B):
        tabp = np.concatenate([Table[i].reshape(NCELL, H), padrow], axis=0)
        in_maps.append({
            "tabp": np.ascontiguousarray(tabp),
            "edge": np.ascontiguousarray(table_edge[i].reshape(NCELL, H)),
            "idx1": np.ascontiguousarray(fins[i]["idx1"]),
            "idx2": np.ascontiguousarray(fins[i]["idx2"]),
            "fidx": np.ascontiguousarray(fins[i]["fidx"]),
            "wrep": wrep,
        })

    import time as _time
    nruns = int(os.environ.get("KERNEL_RUNS", "1"))
    walls = []
    for _ in range(max(nruns, 1)):
        _t0 = _time.time()
        res = run_bass_kernel_spmd(nc, in_maps, core_ids=list(range(B)))
        walls.append(_time.time() - _t0)
    kernel.last_run_walls = walls
    kernel.last_exec_time_ns = res.exec_time_ns

    # host: unsort, bias, softmax, loss
    logits = np.empty((B, P, 4), np.float64)
    for i in range(B):
        lg = res.results[i]["logits"][:, :4].astype(np.float64)
        logits[i, fins[i]["order"]] = lg
    logits += bb.astype(np.float64)[None, None, :]

    mx = logits.max(axis=-1, keepdims=True)
    ex = np.exp(logits - mx)
    sm = ex.sum(axis=-1, keepdims=True)
    probs = (ex / sm).astype(np.float32)
    logp = (logits - mx) - np.log(sm)

    valid = labels >= 0
    lbl = np.where(valid, labels, 0)
    nll = -np.take_along_axis(logp, lbl[..., None].astype(np.int64), axis=-1)[..., 0]
    denom = max(int(valid.sum()), 1)
    loss = np.float32(np.where(valid, nll, 0.0).sum() / denom)

    return loss, probs
